# revision 27
# baseline (speedup 1.0000x reference)
"""AdditiveAttention Trainium2 kernel (8 NeuronCores, data-parallel over batch).

Computation per batch row b:
    q_proj = query[b] @ W1.T                    # [H]
    k_proj = keys[b] @ W2.T                     # [S, H]
    hidden = tanh(q_proj + k_proj)              # [S, H]
    score  = hidden @ V.T                       # [S]
    w      = softmax(score)                     # [S]
    ctx    = w @ keys[b]                        # [K]
returns (ctx [B, K], w [B, S])

Strategy (per core, 4 batch rows):
  - Host pre-casts keys to fp16 (halves HBM traffic; the device math is fp16
    anyway) and pre-transposes the tiny W1/W2/V/query operands, so the device
    spends no time on weight setup.
  - keys (8 MB/core fp16) is DMA'd once into SBUF as [128(s), 512(k)] tiles
    per s-chunk and consumed twice from SBUF: transposed on the PE (keysT is
    needed because the k_proj contraction over k must sit on the partition
    axis) and directly as the moving operand of the context matmul
    (contraction over s).
  - All matmuls/transposes run in fp16 (1 PE cycle/row) with fp32 PSUM
    accumulation; the softmax itself is exact fp32, using GPSIMD
    partition_all_reduce for the cross-partition max/sum.
  - The per-batch softmax+context ("finish") work is software-pipelined into
    the next batch's block stream so the in-order PE queue never stalls on
    the softmax latency chain.
"""

import numpy as np

B, S, K, Q, H = 32, 2048, 512, 1024, 128
N_CORES = 8
BPC = B // N_CORES          # batch rows per core
NCH = S // 128              # 16 s-chunks of 128 per batch row
NBLK = 4                    # s-blocks of 512 per batch row
KC = K // 128               # 4 k-chunks
QC = Q // 128               # 8 q-chunks

_cached_nc = None


def _build():
    from contextlib import ExitStack
    import concourse.bacc as bacc
    import concourse.tile as tile
    from concourse import mybir, bass_isa

    f32 = mybir.dt.float32
    f16 = mybir.dt.float16
    Act = mybir.ActivationFunctionType

    nc = bacc.Bacc("TRN2", target_bir_lowering=False, debug=False)

    keys_in = nc.dram_tensor("keys_in", [BPC, S, K], f16, kind="ExternalInput").ap()
    qt_in = nc.dram_tensor("qt_in", [Q, BPC], f16, kind="ExternalInput").ap()
    w1t_in = nc.dram_tensor("w1t_in", [Q, H], f16, kind="ExternalInput").ap()
    w2t_in = nc.dram_tensor("w2t_in", [K, H], f16, kind="ExternalInput").ap()
    vt2_in = nc.dram_tensor("vt2_in", [H, 2], f16, kind="ExternalInput").ap()
    ident_in = nc.dram_tensor("ident_in", [128, 128], f16, kind="ExternalInput").ap()
    ctx_out = nc.dram_tensor("ctx_out", [BPC, K], f32, kind="ExternalOutput").ap()
    attn_out = nc.dram_tensor("attn_out", [BPC, S], f32, kind="ExternalOutput").ap()

    with tile.TileContext(nc) as tc, ExitStack() as ctx:
        consts = ctx.enter_context(tc.tile_pool(name="consts", bufs=1))
        keys_pool = ctx.enter_context(tc.tile_pool(name="keys", bufs=1))
        kt_pool = ctx.enter_context(tc.tile_pool(name="kt", bufs=6))
        hid_pool = ctx.enter_context(tc.tile_pool(name="hid", bufs=4))
        sm_pool = ctx.enter_context(tc.tile_pool(name="sm", bufs=2))
        pt_ps = ctx.enter_context(tc.tile_pool(name="pt_ps", bufs=3, space="PSUM"))
        kp_ps = ctx.enter_context(tc.tile_pool(name="kp_ps", bufs=2, space="PSUM"))
        sc_ps = ctx.enter_context(tc.tile_pool(name="sc_ps", bufs=2, space="PSUM"))
        ctx_ps = ctx.enter_context(tc.tile_pool(name="ctx_ps", bufs=1, space="PSUM"))

        # ---- constants / weights (host pre-transposed, fp16) ----
        identr = consts.tile([128, 128], f16)
        nc.sync.dma_start(out=identr[:], in_=ident_in[:])
        w1t = consts.tile([128, QC, H], f16)    # [q%128, q//128, h] = W1T
        nc.sync.dma_start(out=w1t[:],
                          in_=w1t_in[:].rearrange("(c p) h -> p c h", p=128))
        w2t = consts.tile([128, KC, H], f16)
        nc.sync.dma_start(out=w2t[:],
                          in_=w2t_in[:].rearrange("(c p) h -> p c h", p=128))
        vt2 = consts.tile([128, 2], f16)
        nc.sync.dma_start(out=vt2[:], in_=vt2_in[:])
        qt = consts.tile([128, QC, BPC], f16)
        nc.sync.dma_start(out=qt[:],
                          in_=qt_in[:].rearrange("(c p) b -> p c b", p=128))

        # ---- keys load: one 512 KB fp16 DMA per (batch row, s-block) ----
        keys_sb = []
        for b in range(BPC):
            kb = keys_pool.tile([128, NCH, K], f16, tag=f"keys{b}")
            keys_sb.append(kb)
            for j in range(NBLK):
                src = keys_in[b, j * 512:(j + 1) * 512, :].rearrange(
                    "(c p) k -> p c k", p=128
                )
                if b == 0 and j == 0:
                    for c in range(4):
                        nc.sync.dma_start(out=kb[:, c, :], in_=src[:, c, :])
                else:
                    nc.sync.dma_start(out=kb[:, 4 * j:4 * j + 4, :], in_=src)

        # ---- main loop (software-pipelined: batch b's softmax/context is
        # emitted after batch b+1's block work so the in-order PE stream never
        # stalls on the softmax latency chain) ----
        def emit_block_kproj(b, j):
            kp = kp_ps.tile([128, 512], f32, tag="kp")
            for kcp in range(2):
                p = pt_ps.tile([128, 1024], f16, tag="pt")
                for half in range(2):
                    kc = 2 * kcp + half
                    for ci in range(4):
                        nc.tensor.transpose(
                            p[:, half * 512 + ci * 128:half * 512 + (ci + 1) * 128],
                            keys_sb[b][:, 4 * j + ci, kc * 128:(kc + 1) * 128],
                            identr[:])
                kt = kt_pool.tile([128, 1024], f16, tag="kt")
                nc.vector.tensor_copy(kt[:], p[:])
                for half in range(2):
                    kc = 2 * kcp + half
                    nc.tensor.matmul(kp[:], w2t[:, kc, :],
                                     kt[:, half * 512:(half + 1) * 512],
                                     start=(kc == 0), stop=(kc == KC - 1))
            return kp

        def emit_block_scores(b, j, kp, score_sb):
            hid = hid_pool.tile([128, 512], f16, tag="hid")
            nc.scalar.activation(hid[:], kp[:], Act.Tanh,
                                 bias=qp_sb[:, b:b + 1], scale=1.0)
            sc = sc_ps.tile([128, 8], f32, tag="sc")
            for ci in range(4):
                nc.tensor.matmul(sc[:, 2 * ci:2 * ci + 2],
                                 hid[:, ci * 128:(ci + 1) * 128], vt2[:],
                                 start=True, stop=True)
            sc_even = sc[:].rearrange("p (c two) -> p c two", two=2)[:, :, 0]
            nc.vector.tensor_copy(score_sb[:, 4 * j:4 * j + 4], sc_even)

        def emit_blocks(b, js, score_sb=None):
            if score_sb is None:
                score_sb = sm_pool.tile([128, NCH], f32, tag="score")
            for j in js:
                kp = emit_block_kproj(b, j)
                emit_block_scores(b, j, kp, score_sb)
            return score_sb

        def emit_finish(b, score_sb):
            # softmax over the 2048 scores of row b (layout [128, 16])
            mx = sm_pool.tile([128, 1], f32, tag="mx")
            nc.vector.tensor_reduce(mx[:], score_sb[:], axis=mybir.AxisListType.X,
                                    op=mybir.AluOpType.max)
            mxr = sm_pool.tile([128, 1], f32, tag="mxr")
            nc.gpsimd.partition_all_reduce(mxr[:], mx[:], channels=128,
                                           reduce_op=bass_isa.ReduceOp.max)
            negmx = sm_pool.tile([128, 1], f32, tag="negmx")
            nc.vector.tensor_scalar_mul(negmx[:], mxr[:], -1.0)
            w_sb = sm_pool.tile([128, NCH], f32, tag="w")
            sums = sm_pool.tile([128, 1], f32, tag="sums")
            nc.scalar.activation(w_sb[:], score_sb[:], Act.Exp,
                                 bias=negmx[:, 0:1], scale=1.0,
                                 accum_out=sums[:, 0:1])
            smr = sm_pool.tile([128, 1], f32, tag="smr")
            nc.gpsimd.partition_all_reduce(smr[:], sums[:], channels=128,
                                           reduce_op=bass_isa.ReduceOp.add)
            rs = sm_pool.tile([128, 1], f32, tag="rs")
            nc.vector.reciprocal(rs[:], smr[:])
            wr = sm_pool.tile([128, NCH], f16, tag="wr")
            nc.vector.tensor_scalar_mul(wr[:], w_sb[:], rs[:, 0:1])

            # context: ctx[b] = sum_s w[s] * keys[b, s, :]
            cps = ctx_ps.tile([1, K], f32, tag="ctx")
            for c in range(NCH):
                nc.tensor.matmul(cps[:], wr[:, c:c + 1], keys_sb[b][:, c, :],
                                 start=(c == 0), stop=(c == NCH - 1))
            ctx_sb = sm_pool.tile([1, K], f32, tag="ctxsb")
            nc.vector.tensor_copy(ctx_sb[:], cps[:])
            nc.sync.dma_start(out=ctx_out[b:b + 1, :], in_=ctx_sb[:])

            # attention weights out: transpose [128, 16] -> [16, 128] on PE,
            # then one contiguous DMA (row c holds s = c*128 + p)
            pw = sc_ps.tile([128, 128], f16, tag="sc")
            nc.tensor.transpose(pw[0:NCH, 0:128], wr[:], identr[:])
            wt_sb = sm_pool.tile([NCH, 128], f32, tag="wt")
            nc.vector.tensor_copy(wt_sb[:], pw[0:NCH, 0:128])
            nc.sync.dma_start(
                out=attn_out[b].rearrange("(c p) -> c p", p=128), in_=wt_sb[:])

        # batch 0/1 keys arrive while the PE is still ramping, so finish(0)
        # hides mid-batch-1; later batches run DMA-bound, so finish(b-1) goes
        # before blocks(b) where its softmax latency hides inside the keys wait
        score_tiles = {}
        # block (0,0) matmul phase first: the PE starts on key transposes,
        # which wait only on the small identity + first keys chunks; q_proj
        # (gated by the slower w1t DMA) is emitted afterwards but before the
        # first tanh that consumes it
        score0 = sm_pool.tile([128, NCH], f32, tag="score")
        kp00 = emit_block_kproj(0, 0)
        qp_psum = kp_ps.tile([128, BPC], f32, tag="kp")
        for qc in range(QC):
            nc.tensor.matmul(qp_psum[:], w1t[:, qc, :], qt[:, qc, :],
                             start=(qc == 0), stop=(qc == QC - 1))
        qp_sb = consts.tile([128, BPC], f32)
        nc.scalar.copy(qp_sb[:], qp_psum[:])
        emit_block_scores(0, 0, kp00, score0)
        score_tiles[0] = emit_blocks(0, range(1, NBLK), score0)
        score_tiles[1] = emit_blocks(1, range(0, 2))
        emit_finish(0, score_tiles[0])
        emit_blocks(1, range(2, NBLK), score_tiles[1])
        emit_finish(1, score_tiles[1])
        score_tiles[2] = emit_blocks(2, range(NBLK))
        emit_finish(2, score_tiles[2])
        score_tiles[3] = emit_blocks(3, range(NBLK))
        emit_finish(3, score_tiles[3])

    nc.compile()
    return nc


def _get_nc():
    global _cached_nc
    if _cached_nc is None:
        _cached_nc = _build()
    return _cached_nc


def run(query, keys, W1, W2, V, **spmd_kwargs):
    from concourse import bass_utils

    query = np.asarray(query, dtype=np.float32)
    keys16 = np.ascontiguousarray(
        np.asarray(keys, dtype=np.float32).astype(np.float16))
    W1 = np.asarray(W1, dtype=np.float32)
    W2 = np.asarray(W2, dtype=np.float32)
    V = np.asarray(V, dtype=np.float32)

    w1t_np = np.ascontiguousarray(W1.T.astype(np.float16))
    w2t_np = np.ascontiguousarray(W2.T.astype(np.float16))
    vt2_np = np.ascontiguousarray(np.repeat(V.T, 2, axis=1).astype(np.float16))
    qt_np = query.T.astype(np.float16)
    ident_np = np.eye(128, dtype=np.float16)

    nc = _get_nc()
    in_maps = []
    for core in range(N_CORES):
        sl = slice(core * BPC, (core + 1) * BPC)
        in_maps.append({
            "keys_in": keys16[sl],
            "qt_in": np.ascontiguousarray(qt_np[:, sl]),
            "w1t_in": w1t_np,
            "w2t_in": w2t_np,
            "vt2_in": vt2_np,
            "ident_in": ident_np,
        })
    res = bass_utils.run_bass_kernel_spmd(
        nc, in_maps, core_ids=list(range(N_CORES)), **spmd_kwargs)
    ctx = np.concatenate([res.results[i]["ctx_out"] for i in range(N_CORES)], axis=0)
    attn = np.concatenate([res.results[i]["attn_out"] for i in range(N_CORES)], axis=0)
    return (ctx, attn), res


def kernel(query, keys, W1, W2, V):
    # one retry: a crashed prior kernel can leave the accelerator in a
    # transiently unrecoverable state that clears after ~a minute
    try:
        (ctx, attn), _ = run(query, keys, W1, W2, V)
    except Exception:
        import time
        time.sleep(75)
        (ctx, attn), _ = run(query, keys, W1, W2, V)
    return ctx, attn



# revision 28
# speedup vs baseline: 1.0483x; 1.0483x over previous
"""AdditiveAttention Trainium2 kernel (8 NeuronCores, data-parallel over batch).

Computation per batch row b:
    q_proj = query[b] @ W1.T                    # [H]
    k_proj = keys[b] @ W2.T                     # [S, H]
    hidden = tanh(q_proj + k_proj)              # [S, H]
    score  = hidden @ V.T                       # [S]
    w      = softmax(score)                     # [S]
    ctx    = w @ keys[b]                        # [K]
returns (ctx [B, K], w [B, S])

Strategy (per core, 4 batch rows):
  - Host pre-casts keys to fp16 (halves HBM traffic; the device math is fp16
    anyway) and pre-transposes the tiny W1/W2/V/query operands, so the device
    spends no time on weight setup.
  - keys (8 MB/core fp16) is DMA'd once into SBUF as [128(s), 512(k)] tiles
    per s-chunk and consumed twice from SBUF: transposed on the PE (keysT is
    needed because the k_proj contraction over k must sit on the partition
    axis) and directly as the moving operand of the context matmul
    (contraction over s).
  - All matmuls/transposes run in fp16 (1 PE cycle/row) with fp32 PSUM
    accumulation; the softmax itself is exact fp32, using GPSIMD
    partition_all_reduce for the cross-partition max/sum.
  - The per-batch softmax+context ("finish") work is software-pipelined into
    the next batch's block stream so the in-order PE queue never stalls on
    the softmax latency chain.
"""

import numpy as np

B, S, K, Q, H = 32, 2048, 512, 1024, 128
N_CORES = 8
BPC = B // N_CORES          # batch rows per core
NCH = S // 128              # 16 s-chunks of 128 per batch row
NBLK = 4                    # s-blocks of 512 per batch row
KC = K // 128               # 4 k-chunks
QC = Q // 128               # 8 q-chunks

_cached_nc = None


def _build():
    from contextlib import ExitStack
    import concourse.bacc as bacc
    import concourse.tile as tile
    from concourse import mybir, bass_isa

    f32 = mybir.dt.float32
    f16 = mybir.dt.float16
    Act = mybir.ActivationFunctionType

    nc = bacc.Bacc("TRN2", target_bir_lowering=False, debug=False)

    keys_in = nc.dram_tensor("keys_in", [BPC, S, K], f16, kind="ExternalInput").ap()
    qt_in = nc.dram_tensor("qt_in", [Q, BPC], f16, kind="ExternalInput").ap()
    w1t_in = nc.dram_tensor("w1t_in", [Q, H], f16, kind="ExternalInput").ap()
    w2t_in = nc.dram_tensor("w2t_in", [K, H], f16, kind="ExternalInput").ap()
    vt2_in = nc.dram_tensor("vt2_in", [H, 2], f16, kind="ExternalInput").ap()
    ident_in = nc.dram_tensor("ident_in", [128, 128], f16, kind="ExternalInput").ap()
    ctx_out = nc.dram_tensor("ctx_out", [BPC, K], f32, kind="ExternalOutput").ap()
    attn_out = nc.dram_tensor("attn_out", [BPC, S], f32, kind="ExternalOutput").ap()

    with tile.TileContext(nc) as tc, ExitStack() as ctx:
        consts = ctx.enter_context(tc.tile_pool(name="consts", bufs=1))
        keys_pool = ctx.enter_context(tc.tile_pool(name="keys", bufs=1))
        kt_pool = ctx.enter_context(tc.tile_pool(name="kt", bufs=6))
        hid_pool = ctx.enter_context(tc.tile_pool(name="hid", bufs=4))
        sm_pool = ctx.enter_context(tc.tile_pool(name="sm", bufs=2))
        pt_ps = ctx.enter_context(tc.tile_pool(name="pt_ps", bufs=3, space="PSUM"))
        kp_ps = ctx.enter_context(tc.tile_pool(name="kp_ps", bufs=2, space="PSUM"))
        sc_ps = ctx.enter_context(tc.tile_pool(name="sc_ps", bufs=2, space="PSUM"))
        ctx_ps = ctx.enter_context(tc.tile_pool(name="ctx_ps", bufs=1, space="PSUM"))

        # ---- constants / weights (host pre-transposed, fp16) ----
        identr = consts.tile([128, 128], f16)
        nc.sync.dma_start(out=identr[:], in_=ident_in[:])
        w1t = consts.tile([128, QC, H], f16)    # [q%128, q//128, h] = W1T
        nc.sync.dma_start(out=w1t[:],
                          in_=w1t_in[:].rearrange("(c p) h -> p c h", p=128))
        w2t = consts.tile([128, KC, H], f16)
        nc.sync.dma_start(out=w2t[:],
                          in_=w2t_in[:].rearrange("(c p) h -> p c h", p=128))
        vt2 = consts.tile([128, 2], f16)
        nc.sync.dma_start(out=vt2[:], in_=vt2_in[:])
        qt = consts.tile([128, QC, BPC], f16)
        nc.sync.dma_start(out=qt[:],
                          in_=qt_in[:].rearrange("(c p) b -> p c b", p=128))

        # ---- keys load: one 512 KB fp16 DMA per (batch row, s-block) ----
        keys_sb = []
        for b in range(BPC):
            kb = keys_pool.tile([128, NCH, K], f16, tag=f"keys{b}")
            keys_sb.append(kb)
            for j in range(NBLK):
                src = keys_in[b, j * 512:(j + 1) * 512, :].rearrange(
                    "(c p) k -> p c k", p=128
                )
                if b == 0 and j == 0:
                    for c in range(4):
                        nc.sync.dma_start(out=kb[:, c, :], in_=src[:, c, :])
                else:
                    nc.sync.dma_start(out=kb[:, 4 * j:4 * j + 4, :], in_=src)

        # ---- main loop (software-pipelined: batch b's softmax/context is
        # emitted after batch b+1's block work so the in-order PE stream never
        # stalls on the softmax latency chain) ----
        def emit_block_kproj(b, j):
            kp = kp_ps.tile([128, 512], f32, tag="kp")
            for kcp in range(2):
                p = pt_ps.tile([128, 1024], f16, tag="pt")
                for half in range(2):
                    kc = 2 * kcp + half
                    for ci in range(4):
                        nc.tensor.transpose(
                            p[:, half * 512 + ci * 128:half * 512 + (ci + 1) * 128],
                            keys_sb[b][:, 4 * j + ci, kc * 128:(kc + 1) * 128],
                            identr[:])
                kt = kt_pool.tile([128, 1024], f16, tag="kt")
                nc.vector.tensor_copy(kt[:], p[:])
                for half in range(2):
                    kc = 2 * kcp + half
                    nc.tensor.matmul(kp[:], w2t[:, kc, :],
                                     kt[:, half * 512:(half + 1) * 512],
                                     start=(kc == 0), stop=(kc == KC - 1))
            return kp

        def emit_block_scores(b, j, kp, score_sb):
            hid = hid_pool.tile([128, 512], f16, tag="hid")
            nc.scalar.activation(hid[:], kp[:], Act.Tanh,
                                 bias=qp_sb[:, b:b + 1], scale=1.0)
            sc = sc_ps.tile([128, 8], f32, tag="sc")
            for ci in range(4):
                nc.tensor.matmul(sc[:, 2 * ci:2 * ci + 2],
                                 hid[:, ci * 128:(ci + 1) * 128], vt2[:],
                                 start=True, stop=True)
            sc_even = sc[:].rearrange("p (c two) -> p c two", two=2)[:, :, 0]
            nc.vector.tensor_copy(score_sb[:, 4 * j:4 * j + 4], sc_even)

        def emit_blocks(b, js, score_sb=None):
            if score_sb is None:
                score_sb = sm_pool.tile([128, NCH], f32, tag="score")
            for j in js:
                kp = emit_block_kproj(b, j)
                emit_block_scores(b, j, kp, score_sb)
            return score_sb

        def emit_finish(b, score_sb):
            # softmax over the 2048 scores of row b (layout [128, 16])
            mx = sm_pool.tile([128, 1], f32, tag="mx")
            nc.vector.tensor_reduce(mx[:], score_sb[:], axis=mybir.AxisListType.X,
                                    op=mybir.AluOpType.max)
            mxr = sm_pool.tile([128, 1], f32, tag="mxr")
            nc.gpsimd.partition_all_reduce(mxr[:], mx[:], channels=128,
                                           reduce_op=bass_isa.ReduceOp.max)
            negmx = sm_pool.tile([128, 1], f32, tag="negmx")
            nc.vector.tensor_scalar_mul(negmx[:], mxr[:], -1.0)
            # unnormalized exp weights: the context matmuls can start right
            # after the exp; the 1/Z normalizer is folded into the f32 output
            # copies and overlaps the sum-allreduce/reciprocal chain
            wr = sm_pool.tile([128, NCH], f16, tag="wr")
            sums = sm_pool.tile([128, 1], f32, tag="sums")
            nc.scalar.activation(wr[:], score_sb[:], Act.Exp,
                                 bias=negmx[:, 0:1], scale=1.0,
                                 accum_out=sums[:, 0:1])
            smr = sm_pool.tile([128, 1], f32, tag="smr")
            nc.gpsimd.partition_all_reduce(smr[:], sums[:], channels=128,
                                           reduce_op=bass_isa.ReduceOp.add)
            rs = sm_pool.tile([128, 1], f32, tag="rs")
            nc.vector.reciprocal(rs[:], smr[:])

            # context: ctx[b] = (sum_s wx[s] * keys[b, s, :]) / Z
            cps = ctx_ps.tile([1, K], f32, tag="ctx")
            for c in range(NCH):
                nc.tensor.matmul(cps[:], wr[:, c:c + 1], keys_sb[b][:, c, :],
                                 start=(c == 0), stop=(c == NCH - 1))
            ctx_sb = sm_pool.tile([1, K], f32, tag="ctxsb")
            nc.scalar.mul(ctx_sb[:], cps[:], rs[0:1, 0:1])
            nc.sync.dma_start(out=ctx_out[b:b + 1, :], in_=ctx_sb[:])

            # attention weights out: transpose [128, 16] -> [16, 128] on PE,
            # scale by 1/Z in the f32 copy, one contiguous DMA
            pw = sc_ps.tile([128, 128], f16, tag="sc")
            nc.tensor.transpose(pw[0:NCH, 0:128], wr[:], identr[:])
            wt_sb = sm_pool.tile([NCH, 128], f32, tag="wt")
            nc.scalar.mul(wt_sb[:], pw[0:NCH, 0:128], rs[0:NCH, 0:1])
            nc.sync.dma_start(
                out=attn_out[b].rearrange("(c p) -> c p", p=128), in_=wt_sb[:])

        # batch 0/1 keys arrive while the PE is still ramping, so finish(0)
        # hides mid-batch-1; later batches run DMA-bound, so finish(b-1) goes
        # before blocks(b) where its softmax latency hides inside the keys wait
        score_tiles = {}
        # block (0,0) matmul phase first: the PE starts on key transposes,
        # which wait only on the small identity + first keys chunks; q_proj
        # (gated by the slower w1t DMA) is emitted afterwards but before the
        # first tanh that consumes it
        score0 = sm_pool.tile([128, NCH], f32, tag="score")
        kp00 = emit_block_kproj(0, 0)
        qp_psum = kp_ps.tile([128, BPC], f32, tag="kp")
        for qc in range(QC):
            nc.tensor.matmul(qp_psum[:], w1t[:, qc, :], qt[:, qc, :],
                             start=(qc == 0), stop=(qc == QC - 1))
        qp_sb = consts.tile([128, BPC], f32)
        nc.scalar.copy(qp_sb[:], qp_psum[:])
        emit_block_scores(0, 0, kp00, score0)
        score_tiles[0] = emit_blocks(0, range(1, NBLK), score0)
        score_tiles[1] = emit_blocks(1, range(0, 2))
        emit_finish(0, score_tiles[0])
        emit_blocks(1, range(2, NBLK), score_tiles[1])
        emit_finish(1, score_tiles[1])
        score_tiles[2] = emit_blocks(2, range(NBLK))
        emit_finish(2, score_tiles[2])
        score_tiles[3] = emit_blocks(3, range(NBLK))
        emit_finish(3, score_tiles[3])

    nc.compile()
    return nc


def _get_nc():
    global _cached_nc
    if _cached_nc is None:
        _cached_nc = _build()
    return _cached_nc


def run(query, keys, W1, W2, V, **spmd_kwargs):
    from concourse import bass_utils

    query = np.asarray(query, dtype=np.float32)
    keys16 = np.ascontiguousarray(
        np.asarray(keys, dtype=np.float32).astype(np.float16))
    W1 = np.asarray(W1, dtype=np.float32)
    W2 = np.asarray(W2, dtype=np.float32)
    V = np.asarray(V, dtype=np.float32)

    w1t_np = np.ascontiguousarray(W1.T.astype(np.float16))
    w2t_np = np.ascontiguousarray(W2.T.astype(np.float16))
    vt2_np = np.ascontiguousarray(np.repeat(V.T, 2, axis=1).astype(np.float16))
    qt_np = query.T.astype(np.float16)
    ident_np = np.eye(128, dtype=np.float16)

    nc = _get_nc()
    in_maps = []
    for core in range(N_CORES):
        sl = slice(core * BPC, (core + 1) * BPC)
        in_maps.append({
            "keys_in": keys16[sl],
            "qt_in": np.ascontiguousarray(qt_np[:, sl]),
            "w1t_in": w1t_np,
            "w2t_in": w2t_np,
            "vt2_in": vt2_np,
            "ident_in": ident_np,
        })
    res = bass_utils.run_bass_kernel_spmd(
        nc, in_maps, core_ids=list(range(N_CORES)), **spmd_kwargs)
    ctx = np.concatenate([res.results[i]["ctx_out"] for i in range(N_CORES)], axis=0)
    attn = np.concatenate([res.results[i]["attn_out"] for i in range(N_CORES)], axis=0)
    return (ctx, attn), res


def kernel(query, keys, W1, W2, V):
    # one retry: a crashed prior kernel can leave the accelerator in a
    # transiently unrecoverable state that clears after ~a minute
    try:
        (ctx, attn), _ = run(query, keys, W1, W2, V)
    except Exception:
        import time
        time.sleep(75)
        (ctx, attn), _ = run(query, keys, W1, W2, V)
    return ctx, attn



# revision 29
# speedup vs baseline: 1.0586x; 1.0098x over previous
"""AdditiveAttention Trainium2 kernel (8 NeuronCores, data-parallel over batch).

Computation per batch row b:
    q_proj = query[b] @ W1.T                    # [H]
    k_proj = keys[b] @ W2.T                     # [S, H]
    hidden = tanh(q_proj + k_proj)              # [S, H]
    score  = hidden @ V.T                       # [S]
    w      = softmax(score)                     # [S]
    ctx    = w @ keys[b]                        # [K]
returns (ctx [B, K], w [B, S])

Strategy (per core, 4 batch rows):
  - Host pre-casts keys to fp16 (halves HBM traffic; the device math is fp16
    anyway) and pre-transposes the tiny W1/W2/V/query operands, so the device
    spends no time on weight setup.
  - keys (8 MB/core fp16) is DMA'd once into SBUF as [128(s), 512(k)] tiles
    per s-chunk and consumed twice from SBUF: transposed on the PE (keysT is
    needed because the k_proj contraction over k must sit on the partition
    axis) and directly as the moving operand of the context matmul
    (contraction over s).
  - All matmuls/transposes run in fp16 (1 PE cycle/row) with fp32 PSUM
    accumulation; the softmax itself is exact fp32, using GPSIMD
    partition_all_reduce for the cross-partition max/sum.
  - The per-batch softmax+context ("finish") work is software-pipelined into
    the next batch's block stream so the in-order PE queue never stalls on
    the softmax latency chain.
"""

import numpy as np

B, S, K, Q, H = 32, 2048, 512, 1024, 128
N_CORES = 8
BPC = B // N_CORES          # batch rows per core
NCH = S // 128              # 16 s-chunks of 128 per batch row
NBLK = 4                    # s-blocks of 512 per batch row
KC = K // 128               # 4 k-chunks
QC = Q // 128               # 8 q-chunks

_cached_nc = None


def _build():
    from contextlib import ExitStack
    import concourse.bacc as bacc
    import concourse.tile as tile
    from concourse import mybir, bass_isa

    f32 = mybir.dt.float32
    f16 = mybir.dt.float16
    Act = mybir.ActivationFunctionType

    nc = bacc.Bacc("TRN2", target_bir_lowering=False, debug=False)

    keys_in = nc.dram_tensor("keys_in", [BPC, S, K], f16, kind="ExternalInput").ap()
    qt_in = nc.dram_tensor("qt_in", [Q, BPC], f16, kind="ExternalInput").ap()
    w1t_in = nc.dram_tensor("w1t_in", [Q, H], f16, kind="ExternalInput").ap()
    w2t_in = nc.dram_tensor("w2t_in", [K, H], f16, kind="ExternalInput").ap()
    vt2_in = nc.dram_tensor("vt2_in", [H, 2], f16, kind="ExternalInput").ap()
    ident_in = nc.dram_tensor("ident_in", [128, 128], f16, kind="ExternalInput").ap()
    ctx_out = nc.dram_tensor("ctx_out", [BPC, K], f32, kind="ExternalOutput").ap()
    attn_out = nc.dram_tensor("attn_out", [BPC, S], f32, kind="ExternalOutput").ap()

    with tile.TileContext(nc) as tc, ExitStack() as ctx:
        consts = ctx.enter_context(tc.tile_pool(name="consts", bufs=1))
        keys_pool = ctx.enter_context(tc.tile_pool(name="keys", bufs=1))
        kt_pool = ctx.enter_context(tc.tile_pool(name="kt", bufs=6))
        hid_pool = ctx.enter_context(tc.tile_pool(name="hid", bufs=4))
        sm_pool = ctx.enter_context(tc.tile_pool(name="sm", bufs=2))
        pt_ps = ctx.enter_context(tc.tile_pool(name="pt_ps", bufs=3, space="PSUM"))
        kp_ps = ctx.enter_context(tc.tile_pool(name="kp_ps", bufs=2, space="PSUM"))
        sc_ps = ctx.enter_context(tc.tile_pool(name="sc_ps", bufs=2, space="PSUM"))
        ctx_ps = ctx.enter_context(tc.tile_pool(name="ctx_ps", bufs=1, space="PSUM"))

        # ---- constants / weights (host pre-transposed, fp16) ----
        identr = consts.tile([128, 128], f16)
        nc.sync.dma_start(out=identr[:], in_=ident_in[:])
        w1t = consts.tile([128, QC, H], f16)    # [q%128, q//128, h] = W1T
        nc.sync.dma_start(out=w1t[:],
                          in_=w1t_in[:].rearrange("(c p) h -> p c h", p=128))
        w2t = consts.tile([128, KC, H], f16)
        nc.sync.dma_start(out=w2t[:],
                          in_=w2t_in[:].rearrange("(c p) h -> p c h", p=128))
        vt2 = consts.tile([128, 2], f16)
        nc.sync.dma_start(out=vt2[:], in_=vt2_in[:])
        qt = consts.tile([128, QC, BPC], f16)
        nc.sync.dma_start(out=qt[:],
                          in_=qt_in[:].rearrange("(c p) b -> p c b", p=128))

        # ---- HAM warmup: the PE idles ~7us waiting for the first keys
        # chunk, which lets the clock gate re-throttle to 1.2 GHz; these
        # result-unused matmuls on the identity keep it at 2.4 GHz ----
        warm_ps = kp_ps.tile([128, 128], f32, tag="kp")
        for _ in range(48):
            nc.tensor.matmul(warm_ps[:], identr[:], identr[:],
                             start=True, stop=True)

        # ---- keys load: one 512 KB fp16 DMA per (batch row, s-block) ----
        keys_sb = []
        for b in range(BPC):
            kb = keys_pool.tile([128, NCH, K], f16, tag=f"keys{b}")
            keys_sb.append(kb)
            for j in range(NBLK):
                src = keys_in[b, j * 512:(j + 1) * 512, :].rearrange(
                    "(c p) k -> p c k", p=128
                )
                if b == 0 and j == 0:
                    for c in range(4):
                        nc.sync.dma_start(out=kb[:, c, :], in_=src[:, c, :])
                else:
                    nc.sync.dma_start(out=kb[:, 4 * j:4 * j + 4, :], in_=src)

        # ---- main loop (software-pipelined: batch b's softmax/context is
        # emitted after batch b+1's block work so the in-order PE stream never
        # stalls on the softmax latency chain) ----
        def emit_block_kproj(b, j):
            kp = kp_ps.tile([128, 512], f32, tag="kp")
            for kcp in range(2):
                p = pt_ps.tile([128, 1024], f16, tag="pt")
                for half in range(2):
                    kc = 2 * kcp + half
                    for ci in range(4):
                        nc.tensor.transpose(
                            p[:, half * 512 + ci * 128:half * 512 + (ci + 1) * 128],
                            keys_sb[b][:, 4 * j + ci, kc * 128:(kc + 1) * 128],
                            identr[:])
                kt = kt_pool.tile([128, 1024], f16, tag="kt")
                nc.vector.tensor_copy(kt[:], p[:])
                for half in range(2):
                    kc = 2 * kcp + half
                    nc.tensor.matmul(kp[:], w2t[:, kc, :],
                                     kt[:, half * 512:(half + 1) * 512],
                                     start=(kc == 0), stop=(kc == KC - 1))
            return kp

        def emit_block_scores(b, j, kp, score_sb):
            hid = hid_pool.tile([128, 512], f16, tag="hid")
            nc.scalar.activation(hid[:], kp[:], Act.Tanh,
                                 bias=qp_sb[:, b:b + 1], scale=1.0)
            sc = sc_ps.tile([128, 8], f32, tag="sc")
            for ci in range(4):
                nc.tensor.matmul(sc[:, 2 * ci:2 * ci + 2],
                                 hid[:, ci * 128:(ci + 1) * 128], vt2[:],
                                 start=True, stop=True)
            sc_even = sc[:].rearrange("p (c two) -> p c two", two=2)[:, :, 0]
            nc.vector.tensor_copy(score_sb[:, 4 * j:4 * j + 4], sc_even)

        def emit_blocks(b, js, score_sb=None):
            if score_sb is None:
                score_sb = sm_pool.tile([128, NCH], f32, tag="score")
            for j in js:
                kp = emit_block_kproj(b, j)
                emit_block_scores(b, j, kp, score_sb)
            return score_sb

        def emit_finish(b, score_sb):
            # softmax over the 2048 scores of row b (layout [128, 16])
            mx = sm_pool.tile([128, 1], f32, tag="mx")
            nc.vector.tensor_reduce(mx[:], score_sb[:], axis=mybir.AxisListType.X,
                                    op=mybir.AluOpType.max)
            mxr = sm_pool.tile([128, 1], f32, tag="mxr")
            nc.gpsimd.partition_all_reduce(mxr[:], mx[:], channels=128,
                                           reduce_op=bass_isa.ReduceOp.max)
            negmx = sm_pool.tile([128, 1], f32, tag="negmx")
            nc.vector.tensor_scalar_mul(negmx[:], mxr[:], -1.0)
            # unnormalized exp weights: the context matmuls can start right
            # after the exp; the 1/Z normalizer is folded into the f32 output
            # copies and overlaps the sum-allreduce/reciprocal chain
            wr = sm_pool.tile([128, NCH], f16, tag="wr")
            sums = sm_pool.tile([128, 1], f32, tag="sums")
            nc.scalar.activation(wr[:], score_sb[:], Act.Exp,
                                 bias=negmx[:, 0:1], scale=1.0,
                                 accum_out=sums[:, 0:1])
            smr = sm_pool.tile([128, 1], f32, tag="smr")
            nc.gpsimd.partition_all_reduce(smr[:], sums[:], channels=128,
                                           reduce_op=bass_isa.ReduceOp.add)
            rs = sm_pool.tile([128, 1], f32, tag="rs")
            nc.vector.reciprocal(rs[:], smr[:])

            # attention weights out first (needs only wr): transpose
            # [128, 16] -> [16, 128] on PE, scale by 1/Z in the f32 copy,
            # one contiguous DMA that overlaps the context matmul chain
            pw = sc_ps.tile([128, 128], f16, tag="sc")
            nc.tensor.transpose(pw[0:NCH, 0:128], wr[:], identr[:])
            wt_sb = sm_pool.tile([NCH, 128], f32, tag="wt")
            nc.scalar.mul(wt_sb[:], pw[0:NCH, 0:128], rs[0:NCH, 0:1])
            nc.sync.dma_start(
                out=attn_out[b].rearrange("(c p) -> c p", p=128), in_=wt_sb[:])

            # context: ctx[b] = (sum_s wx[s] * keys[b, s, :]) / Z
            cps = ctx_ps.tile([1, K], f32, tag="ctx")
            for c in range(NCH):
                nc.tensor.matmul(cps[:], wr[:, c:c + 1], keys_sb[b][:, c, :],
                                 start=(c == 0), stop=(c == NCH - 1))
            ctx_sb = sm_pool.tile([1, K], f32, tag="ctxsb")
            nc.scalar.mul(ctx_sb[:], cps[:], rs[0:1, 0:1])
            nc.sync.dma_start(out=ctx_out[b:b + 1, :], in_=ctx_sb[:])

        # batch 0/1 keys arrive while the PE is still ramping, so finish(0)
        # hides mid-batch-1; later batches run DMA-bound, so finish(b-1) goes
        # before blocks(b) where its softmax latency hides inside the keys wait
        score_tiles = {}
        # block (0,0) matmul phase first: the PE starts on key transposes,
        # which wait only on the small identity + first keys chunks; q_proj
        # (gated by the slower w1t DMA) is emitted afterwards but before the
        # first tanh that consumes it
        score0 = sm_pool.tile([128, NCH], f32, tag="score")
        kp00 = emit_block_kproj(0, 0)
        qp_psum = kp_ps.tile([128, BPC], f32, tag="kp")
        for qc in range(QC):
            nc.tensor.matmul(qp_psum[:], w1t[:, qc, :], qt[:, qc, :],
                             start=(qc == 0), stop=(qc == QC - 1))
        qp_sb = consts.tile([128, BPC], f32)
        nc.scalar.copy(qp_sb[:], qp_psum[:])
        emit_block_scores(0, 0, kp00, score0)
        score_tiles[0] = emit_blocks(0, range(1, NBLK), score0)
        score_tiles[1] = emit_blocks(1, range(0, 2))
        emit_finish(0, score_tiles[0])
        emit_blocks(1, range(2, NBLK), score_tiles[1])
        emit_finish(1, score_tiles[1])
        score_tiles[2] = emit_blocks(2, range(NBLK))
        emit_finish(2, score_tiles[2])
        score_tiles[3] = emit_blocks(3, range(NBLK))
        emit_finish(3, score_tiles[3])

    nc.compile()
    return nc


def _get_nc():
    global _cached_nc
    if _cached_nc is None:
        _cached_nc = _build()
    return _cached_nc


def run(query, keys, W1, W2, V, **spmd_kwargs):
    from concourse import bass_utils

    query = np.asarray(query, dtype=np.float32)
    keys16 = np.ascontiguousarray(
        np.asarray(keys, dtype=np.float32).astype(np.float16))
    W1 = np.asarray(W1, dtype=np.float32)
    W2 = np.asarray(W2, dtype=np.float32)
    V = np.asarray(V, dtype=np.float32)

    w1t_np = np.ascontiguousarray(W1.T.astype(np.float16))
    w2t_np = np.ascontiguousarray(W2.T.astype(np.float16))
    vt2_np = np.ascontiguousarray(np.repeat(V.T, 2, axis=1).astype(np.float16))
    qt_np = query.T.astype(np.float16)
    ident_np = np.eye(128, dtype=np.float16)

    nc = _get_nc()
    in_maps = []
    for core in range(N_CORES):
        sl = slice(core * BPC, (core + 1) * BPC)
        in_maps.append({
            "keys_in": keys16[sl],
            "qt_in": np.ascontiguousarray(qt_np[:, sl]),
            "w1t_in": w1t_np,
            "w2t_in": w2t_np,
            "vt2_in": vt2_np,
            "ident_in": ident_np,
        })
    res = bass_utils.run_bass_kernel_spmd(
        nc, in_maps, core_ids=list(range(N_CORES)), **spmd_kwargs)
    ctx = np.concatenate([res.results[i]["ctx_out"] for i in range(N_CORES)], axis=0)
    attn = np.concatenate([res.results[i]["attn_out"] for i in range(N_CORES)], axis=0)
    return (ctx, attn), res


def kernel(query, keys, W1, W2, V):
    # one retry: a crashed prior kernel can leave the accelerator in a
    # transiently unrecoverable state that clears after ~a minute
    try:
        (ctx, attn), _ = run(query, keys, W1, W2, V)
    except Exception:
        import time
        time.sleep(75)
        (ctx, attn), _ = run(query, keys, W1, W2, V)
    return ctx, attn

